# revision 1
# baseline (speedup 1.0000x reference)
"""Trainium2 Bass kernel for nn_MultiHeadAttention (B=2, S=2048, D=1024, H=16).

Sharding (8 cores): batch (2-way) x head-group (4-way).
Core c: batch b=c//4, head-group hg=c%4 (4 heads = 256 of d_model).
Megatron style: Wq/Wk/Wv column-parallel, Wo row-parallel; the 4 partial
outputs per batch are summed on the host (plus b_o).

Per-core device pipeline (all matmuls f32r = TF32-like, 1 cyc/row):
  phase 1: project qhT/khT [do,t] and vh [t,do] from host-pre-transposed
           qT/kT/vT chunks; b_q/b_k fused as per-partition DVE adds, b_v
           commuted to a host-side b_v @ w_o output correction (exact).
  phase 2: per 512-query chunk x head-pair: scoresT[kj,qi] via K=64
           matmuls packed 2-heads-per-PE-pass (tile_position row strips),
           exp on ACT (scale 1/8 folded, one FD=2048 op per j-pair,
           fp16 out), keep-mask multiply on DVE (fp16 2x mode), PV
           accumulation with an appended ones
           column so row-sums ride along; normalization stays in [do, t]
           orientation: reciprocal of the sums row, K=1 outer-product
           broadcast across partitions on PE, one TT multiply.
  phase 3: o-proj into natural [t, d_model] layout, DMA out.
"""
import os

if "JAX_PLATFORMS" in os.environ and "axon" not in os.environ["JAX_PLATFORMS"]:
    del os.environ["JAX_PLATFORMS"]

import numpy as np
import ml_dtypes

B, S, D = 2, 2048, 1024
H, DK = 16, 64
NCORES = 8
HGROUPS = 4               # head-groups (cores per batch)
DLOC = D // HGROUPS       # 256 dims per core
NHL = DLOC // DK          # 4 local heads
NKT = D // 128            # 8 k-tiles over d_model
TCH = 512                 # token chunk
NCH = S // TCH            # 4 chunks
NT = S // 128             # 16 token tiles
NKJ = S // 128            # 16 key tiles
SCALE = 1.0 / 8.0         # 1/sqrt(DK)

_CACHE = {}


def _build(reps=1, parts=15):
    """Trace + compile the per-core Bass kernel (cached).

    reps>1 wraps the whole body in a tc.For_i hardware loop (timing use).
    parts: bitmask 1=phase1, 2=attention, 4=finalize, 8=oproj (bisection).
    """
    key = ("nc", reps, parts)
    if key in _CACHE:
        return _CACHE[key]
    import concourse.bacc as bacc
    import concourse.bass as bass
    import concourse.mybir as mybir
    from concourse.tile import TileContext

    f32r = mybir.dt.float32r
    f32 = mybir.dt.float32
    f16 = mybir.dt.float16
    AF = mybir.ActivationFunctionType

    nc = bacc.Bacc("TRN2", target_bir_lowering=False)

    qT_d = nc.dram_tensor("qT", [D, S], f32r, kind="ExternalInput")
    kT_d = nc.dram_tensor("kT", [D, S], f32r, kind="ExternalInput")
    vT_d = nc.dram_tensor("vT", [D, S], f32r, kind="ExternalInput")
    mk_d = nc.dram_tensor("maskT", [S, S], f16, kind="ExternalInput")
    wq_d = nc.dram_tensor("wq", [D, DLOC], f32r, kind="ExternalInput")
    wk_d = nc.dram_tensor("wk", [D, DLOC], f32r, kind="ExternalInput")
    wv_d = nc.dram_tensor("wv", [D, DLOC], f32r, kind="ExternalInput")
    wo_d = nc.dram_tensor("wo", [DLOC, D], f32r, kind="ExternalInput")
    bq_d = nc.dram_tensor("bq", [128, 2], f32r, kind="ExternalInput")
    bk_d = nc.dram_tensor("bk", [128, 2], f32r, kind="ExternalInput")
    ones2_d = nc.dram_tensor("ones2", [128, NT, NHL, 2], f16,
                             kind="ExternalInput")
    onesc_d = nc.dram_tensor("onesc", [1, DK], f32r, kind="ExternalInput")
    out_d = nc.dram_tensor("out", [S, D], f32, kind="ExternalOutput")

    qT_r = qT_d.rearrange("(kt p) t -> p kt t", p=128)
    kT_r = kT_d.rearrange("(kt p) t -> p kt t", p=128)
    vT_r = vT_d.rearrange("(kt p) t -> p kt t", p=128)
    mk_r = mk_d.rearrange("(j p) q -> p j q", p=128)

    with TileContext(nc) as tc:
        with (
            tc.tile_pool(name="big", bufs=1) as big,
            tc.tile_pool(name="xin", bufs=2) as xin,
            tc.tile_pool(name="mp", bufs=2) as mp,
            tc.tile_pool(name="ep", bufs=3) as ep,
            tc.tile_pool(name="sp", bufs=3) as sp,
            tc.tile_pool(name="ps", bufs=1, space="PSUM") as ps,
        ):
          import contextlib
          loop_cm = tc.For_i(0, reps, 1) if reps > 1 else contextlib.nullcontext()
          with loop_cm:
            # ---- constants / weights ----
            wq_sb = big.tile([128, NKT, DLOC], f32r)
            wk_sb = big.tile([128, NKT, DLOC], f32r)
            wv_sb = big.tile([128, NKT, DLOC], f32r)
            wo_sb = big.tile([128, DLOC // 128, D], f32r)
            nc.sync.dma_start(out=wq_sb, in_=wq_d.rearrange("(kt p) o -> p kt o", p=128))
            nc.sync.dma_start(out=wk_sb, in_=wk_d.rearrange("(kt p) o -> p kt o", p=128))
            nc.sync.dma_start(out=wv_sb, in_=wv_d.rearrange("(kt p) o -> p kt o", p=128))
            nc.sync.dma_start(out=wo_sb, in_=wo_d.rearrange("(kk p) o -> p kk o", p=128))
            bq_sb = big.tile([128, 2], f32r)
            bk_sb = big.tile([128, 2], f32r)
            onesc_sb = big.tile([1, DK], f32r)
            nc.sync.dma_start(out=bq_sb, in_=bq_d[:, :])
            nc.sync.dma_start(out=bk_sb, in_=bk_d[:, :])
            nc.sync.dma_start(out=onesc_sb, in_=onesc_d[:, :])

            # ---- persistent activations ----
            qhT_sb = big.tile([128, 2, S], f32r)     # [p, m, t]
            khT_sb = big.tile([128, 2, S], f32r)
            vh1_sb = big.tile([128, NT, NHL, DK + 2], f16)
            aoT_sb = big.tile([128, 2, S], f32r)     # normalized attnout^T
            nc.sync.dma_start(out=vh1_sb[:, :, :, DK:DK + 2], in_=ones2_d[:, :, :, :])

            # one 4-bank psum slot shared by qk-proj (quadrants) and scores
            s4 = ps.tile([128, 2, 2, TCH], f32, tag="s4", name="s4", bufs=1)

            # ---- phase 1: projections ----
            quad = 0

            def emit_qk_chunk(xname, xr, w_sb, b_sb, hT_sb, tch):
                nonlocal quad
                xt = xin.tile([128, NKT, TCH], f32r, tag="xt",
                              name=f"xt_{xname}{tch}")
                nc.sync.dma_start(
                    out=xt, in_=xr[:, :, tch * TCH:(tch + 1) * TCH])
                for m in range(2):
                    acc = s4[:, quad % 2, quad // 2 % 2, :]
                    for kt in range(NKT):
                        nc.tensor.matmul(
                            acc, w_sb[:, kt, m * 128:(m + 1) * 128],
                            xt[:, kt, :],
                            start=(kt == 0), stop=(kt == NKT - 1))
                    nc.vector.tensor_scalar_add(
                        out=hT_sb[:, m, tch * TCH:(tch + 1) * TCH],
                        in0=acc, scalar1=b_sb[:, m:m + 1].bitcast(f32))
                    quad += 1

            def emit_v_chunk(tch):
                xt = xin.tile([128, NKT, TCH], f32r, tag="xt", name=f"xt_v{tch}")
                nc.sync.dma_start(
                    out=xt, in_=vT_r[:, :, tch * TCH:(tch + 1) * TCH])
                for mm in range(TCH // 128):
                    m16 = tch * (TCH // 128) + mm
                    pv = ps.tile([128, DLOC], f32, tag="pvx",
                                 name=f"psv_{m16}", bufs=2)
                    for kt in range(NKT):
                        nc.tensor.matmul(
                            pv, xt[:, kt, mm * 128:(mm + 1) * 128],
                            wv_sb[:, kt, :],
                            start=(kt == 0), stop=(kt == NKT - 1))
                    nc.vector.tensor_copy(
                        vh1_sb[:, m16, :, 0:DK],
                        pv.rearrange("p (h d) -> p h d", h=NHL))

            if parts & 1:
                for tch in range(NCH):
                    emit_qk_chunk("k", kT_r, wk_sb, bk_sb, khT_sb, tch)
                    emit_v_chunk(tch)
                for tch in range(NCH):
                    emit_qk_chunk("q", qT_r, wq_sb, bq_sb, qhT_sb, tch)

            # ---- phase 2: attention ----
            LOOK = 2
            for tcq in range(NCH if parts & 2 else 0):
                qsl = slice(tcq * TCH, (tcq + 1) * TCH)
                mk_sb = mp.tile([128, NKJ, TCH], f16, tag="mk",
                                name=f"mk_{tcq}")
                nc.sync.dma_start(out=mk_sb, in_=mk_r[:, :, tcq * TCH:(tcq + 1) * TCH])
                for hp in range(2):
                    pvT2 = ps.tile([DK + 2, 2, TCH], f32, tag="pvx",
                                   name=f"pvT_{tcq}_{hp}", bufs=2)
                    s_sl = ps.tile([128, 2, 2, TCH], f32, tag="s4",
                                   name=f"s_{tcq}_{hp}", bufs=1)
                    e_tiles = {}
                    NJG = NKJ // 2
                    for jg in range(NJG + 1):
                        if jg < NJG:
                            e_sb = ep.tile([128, 2, 2, TCH], f16, tag="e",
                                           name=f"e_{tcq}_{hp}_{jg}", bufs=3)
                            e_tiles[jg] = e_sb
                            for jj in range(2):
                                j = jg * 2 + jj
                                for hh in range(2):
                                    nc.tensor.matmul(
                                        s_sl[:, jj, hh, :],
                                        khT_sb[64 * hh:64 * (hh + 1), hp,
                                               j * 128:(j + 1) * 128],
                                        qhT_sb[64 * hh:64 * (hh + 1), hp, qsl],
                                        start=True, stop=True,
                                        tile_position=(64 * hh, 0))
                            nc.scalar.activation(
                                out=e_sb, in_=s_sl,
                                func=AF.Exp, scale=SCALE)
                            msl = mk_sb[:, jg * 2:jg * 2 + 2, :]
                            mbc = bass.AP(
                                tensor=msl.tensor, offset=msl.offset,
                                ap=[msl.ap[0], msl.ap[1], [0, 2],
                                    msl.ap[2]])
                            nc.vector.tensor_mul(e_sb, e_sb, mbc)
                        jp = jg - 1
                        if jp >= 0:
                            e_c = e_tiles.pop(jp)
                            for jj in range(2):
                                jc = jp * 2 + jj
                                for hh in range(2):
                                    nc.tensor.matmul(
                                        pvT2[:, hh, :],
                                        vh1_sb[:, jc, hp * 2 + hh, :],
                                        e_c[:, jj, hh, :],
                                        start=(jc == 0),
                                        stop=(jc == NKJ - 1))
                    # finalize pair: normalize in [do, t] orientation:
                    # recip of the sums row, PE-broadcast across partitions
                    # (K=1 outer product), one TT multiply into aoT_sb.
                    if parts & 4:
                        pvT_sb = sp.tile([DK + 2, 2, TCH], f32r, tag="pvs",
                                         name=f"pvs_{tcq}_{hp}")
                        nc.vector.tensor_copy(pvT_sb, pvT2)
                        rec = sp.tile([1, 2, TCH], f32r, tag="rec",
                                      name=f"rec_{tcq}_{hp}")
                        with nc.allow_low_precision(
                                reason="recip row feeds f32r broadcast mm"):
                            nc.vector.reciprocal(rec, pvT_sb[DK:DK + 1, :, :])
                        for hh in range(2):
                            bc = ps.tile([DK, TCH], f32, tag="pvx",
                                         name=f"bc_{tcq}_{hp}_{hh}", bufs=2)
                            nc.tensor.matmul(bc, onesc_sb, rec[0:1, hh, :],
                                             start=True, stop=True)
                            nc.vector.tensor_mul(
                                aoT_sb[64 * hh:64 * (hh + 1), hp, qsl],
                                pvT_sb[0:DK, hh, :], bc)

            # ---- phase 3: o-proj ----
            for m16 in range(NT if parts & 8 else 0):
                o_sb = sp.tile([128, D], f32, tag="o", name=f"o_{m16}")
                po = ps.tile([128, 2, 512], f32, tag="pvx",
                             name=f"po_{m16}", bufs=2)
                for n in range(2):
                    for kk in range(2):
                        nc.tensor.matmul(
                            po[:, n, :],
                            aoT_sb[:, kk, m16 * 128:(m16 + 1) * 128],
                            wo_sb[:, kk, n * 512:(n + 1) * 512],
                            start=(kk == 0), stop=(kk == 1))
                nc.vector.tensor_copy(o_sb.rearrange("p (n q) -> p n q", n=2), po)
                nc.sync.dma_start(
                    out=out_d[m16 * 128:(m16 + 1) * 128, :], in_=o_sb)

    nc.compile()
    _CACHE[key] = nc
    return nc


def _in_maps(q, k, v, mask, w_q, b_q, w_k, b_k, w_v, b_v, w_o, b_o):
    q = np.asarray(q, dtype=np.float32)
    k = np.asarray(k, dtype=np.float32)
    v = np.asarray(v, dtype=np.float32)
    mask = np.asarray(mask)
    w_q = np.asarray(w_q, dtype=np.float32)
    w_k = np.asarray(w_k, dtype=np.float32)
    w_v = np.asarray(w_v, dtype=np.float32)
    w_o = np.asarray(w_o, dtype=np.float32)
    b_q = np.asarray(b_q, dtype=np.float32)
    b_k = np.asarray(b_k, dtype=np.float32)
    b_v = np.asarray(b_v, dtype=np.float32)

    hf = np.float16
    qT = [np.ascontiguousarray(q[b].T) for b in range(B)]
    kT = [np.ascontiguousarray(k[b].T) for b in range(B)]
    vT = [np.ascontiguousarray(v[b].T) for b in range(B)]
    mkT = [np.ascontiguousarray((~mask[b, 0]).T).astype(hf) for b in range(B)]
    ones2 = np.ones((128, NT, NHL, 2), dtype=hf)

    maps = []
    for c in range(NCORES):
        b, hg = c // HGROUPS, c % HGROUPS
        sl = slice(hg * DLOC, (hg + 1) * DLOC)
        maps.append({
            "qT": qT[b], "kT": kT[b], "vT": vT[b], "maskT": mkT[b],
            "wq": np.ascontiguousarray(w_q[:, sl]),
            "wk": np.ascontiguousarray(w_k[:, sl]),
            "wv": np.ascontiguousarray(w_v[:, sl]),
            "wo": np.ascontiguousarray(w_o[sl, :]),
            "bq": np.ascontiguousarray(b_q[sl].reshape(2, 128).T),
            "bk": np.ascontiguousarray(b_k[sl].reshape(2, 128).T),
            "ones2": ones2,
            "onesc": np.ones((1, DK), dtype=np.float32),
        })
    return maps


def kernel(q, k, v, mask, w_q, b_q, w_k, b_k, w_v, b_v, w_o, b_o):
    from concourse.bass_utils import run_bass_kernel_spmd

    nc = _build()
    maps = _in_maps(q, k, v, mask, w_q, b_q, w_k, b_k, w_v, b_v, w_o, b_o)
    res = run_bass_kernel_spmd(nc, maps, list(range(NCORES)))
    b_o = np.asarray(b_o, dtype=np.float32)
    out = np.zeros((B, S, D), dtype=np.float32)
    for c in range(NCORES):
        out[c // HGROUPS] += res.results[c]["out"]
    out += b_o + (np.asarray(b_v, dtype=np.float32) @
                  np.asarray(w_o, dtype=np.float32))
    return out

